# revision 13
# baseline (speedup 1.0000x reference)
import sys

if "/opt/trn_rl_repo" not in sys.path:
    sys.path.insert(0, "/opt/trn_rl_repo")

import numpy as np

from concourse import bass, bacc, mybir
from concourse import tile
from concourse import bass_utils

FP = mybir.dt.float32
I32 = mybir.dt.int32
AF = mybir.ActivationFunctionType
ALU = mybir.AluOpType

NCORES = 8
H = 8
DH = 16
HD = 128

# feature flags (validated in the small test)
BC3 = True        # 3D step-0 broadcast for per-head ops
TDMA_STORE = False # transpose-DMA for SBUF->DRAM row stores
TDMA_LOAD = False  # transpose-DMA for DRAM->SBUF column loads


def _ceil(a, b):
    return (a + b - 1) // b


def _pad512(n):
    return _ceil(n, 512) * 512


def _padblk(n):
    # covers both the 128-dst blocks and 512-wide loops
    return max(_ceil(n, 128) * 128, _pad512(n))


def _sched_edges(src, dst, shard, ncores):
    """Unified (SPMD-identical) chunk schedule: edges partitioned by dst shard,
    sorted by dst, grouped into 128-dst blocks, chunked by 128 edges."""
    nblk = _ceil(shard, 128)
    per_core = []
    counts = np.zeros((ncores, nblk), np.int64)
    for c in range(ncores):
        lo = c * shard
        m = (dst >= lo) & (dst < lo + shard)
        s = src[m].astype(np.int64)
        d = (dst[m] - lo).astype(np.int64)
        o = np.argsort(d, kind="stable")
        s, d = s[o], d[o]
        b = d >> 7
        idx = np.searchsorted(b, np.arange(nblk + 1))
        per_core.append((s, d, idx))
        counts[c] = idx[1:] - idx[:-1]
    nch_blk = np.maximum(1, -(-counts.max(axis=0) // 128)).astype(int)
    total = int(nch_blk.sum())
    metas = np.zeros((ncores, total * 128, 8), np.int32)
    metas[:, :, 2] = 300  # pad sentinel for dst_rel
    sched = []
    cur = 0
    for bi in range(nblk):
        nch = int(nch_blk[bi])
        sched.append((bi, cur, nch))
        for c in range(ncores):
            s, d, idx = per_core[c]
            e0, e1 = int(idx[bi]), int(idx[bi + 1])
            cnt = e1 - e0
            r = slice(cur * 128, cur * 128 + cnt)
            metas[c, r, 0] = s[e0:e1]
            metas[c, r, 1] = d[e0:e1] + c * shard  # global dst row (L1 ad arrays)
            metas[c, r, 2] = d[e0:e1] - bi * 128   # position within block
            metas[c, r, 3] = d[e0:e1]              # local dst row (L2 ad arrays)
        cur += nch
    return metas, sched


def _aseg(a_list):
    """[128, nsets*32]: set si occupies output partitions si*32..si*32+8
    (32-stride keeps engine partition windows on legal boundaries)."""
    out = np.zeros((128, len(a_list) * 32), np.float32)
    for si, a in enumerate(a_list):
        for h in range(H):
            out[h * DH:(h + 1) * DH, si * 32 + h] = a[h]
    return out


def _hmul(nc, out_ap, in_ap, w_ap, pool):
    """out[:, h*16:(h+1)*16] = in[:, h*16:...] * w[:, h] for h in 0..7."""
    if BC3:
        nc.vector.tensor_tensor(
            out=out_ap.rearrange("p (g d) -> p g d", g=H),
            in0=in_ap.rearrange("p (g d) -> p g d", g=H),
            in1=w_ap.rearrange("p (g o) -> p g o", o=1).to_broadcast((128, H, DH)),
            op=ALU.mult,
        )
    else:
        for h in range(H):
            nc.vector.tensor_scalar_mul(
                out_ap[:, h * DH:(h + 1) * DH],
                in_ap[:, h * DH:(h + 1) * DH],
                w_ap[:, h:h + 1],
            )


def _store_rows(nc, sb, nw, dram_rows, ident, sbuf, psum):
    """Store sb[:K, :nw] (feature-major) to dram_rows[:nw, :K] (row-major)."""
    K = sb.shape[0]
    if TDMA_STORE:
        nc.sync.dma_start(out=dram_rows[:nw, :K], in_=sb[:K, :nw], transpose=True)
    else:
        if K < 128:
            # matmul operands need base partition 0/32/64; restage the slice
            stg = sbuf.tile([32, sb.shape[1]], FP, tag="st_stage")
            nc.scalar.copy(stg[:K, :nw], sb[:K, :nw])
            sb = stg
        for j in range(0, nw, 128):
            w = min(128, nw - j)
            tp = psum.tile([128, max(K, 1)], FP, tag="st_tp")
            nc.tensor.transpose(tp[:w, :K], sb[:K, j:j + w], ident[:K, :K])
            tsb = sbuf.tile([128, max(K, 1)], FP, tag="st_sb")
            nc.scalar.copy(tsb[:w, :K], tp[:w, :K])
            nc.sync.dma_start(out=dram_rows[j:j + w, :K], in_=tsb[:w, :K])


def _load_cols(nc, dram_rows, nw, K, sbuf, psum, ident, tag, ptag=None):
    """Load dram_rows[:nw, :K] into an SBUF tile [K, nw] (feature-major)."""
    t = sbuf.tile([128, max(512, nw)], FP, tag=tag)
    if TDMA_LOAD:
        # fp32 DMA transpose writes at most 64 partitions per transfer
        for p0 in range(0, K, 64):
            p1 = min(p0 + 64, K)
            nc.sync.dma_start(out=t[p0:p1, :nw], in_=dram_rows[:nw, p0:p1],
                              transpose=True)
    else:
        for j in range(0, nw, 128):
            w = min(128, nw - j)
            r = sbuf.tile([128, 128], FP, tag=tag + "_r")
            nc.sync.dma_start(out=r[:w, :K], in_=dram_rows[j:j + w, :K])
            tp = psum.tile([128, 128], FP, tag=(ptag or tag + "_tp"))
            nc.tensor.transpose(tp[:K, :w], r[:w, :K], ident)
            nc.scalar.copy(t[:K, j:j + w], tp[:K, :w])
    return t


def build_program(nc, cfg, scheds):
    Nt, Nu, Nm = cfg["Nt"], cfg["Nu"], cfg["Nm"]
    St, Su, Sm = Nt // NCORES, Nu // NCORES, Nm // NCORES
    NtP, NuP, NmP = _padblk(Nt), _padblk(Nu), _padblk(Nm)
    StP, SuP, SmP = _padblk(St), _padblk(Su), _padblk(Sm)
    pad_t = StP - St

    din = {}

    def dram_in(name, shape, dt=FP):
        din[name] = nc.dram_tensor(name, list(shape), dt, kind="ExternalInput")
        return din[name]

    # ---- staged inputs ----
    xT = {k: dram_in(f"xT_{k}", [128, n]) for k, n in
          (("t", NtP), ("u", NuP), ("m", NmP))}
    meta = {et: dram_in(f"meta_{et}", [scheds[et][1], 8], I32)
            for et in ("ut", "mt", "tu", "tm")}
    W1 = {k: dram_in(f"W1_{k}", [128, 128]) for k in "tum"}
    b1 = {k: dram_in(f"b1_{k}", [128, 1]) for k in "tum"}
    A1 = {"t": dram_in("A1_t", [128, 128]), "u": dram_in("A1_u", [128, 64]),
          "m": dram_in("A1_m", [128, 64])}
    W2 = {k: dram_in(f"W2_{k}", [128, 128]) for k in "tum"}
    b2 = {k: dram_in(f"b2_{k}", [128, 1]) for k in "tum"}
    A2 = {"t": dram_in("A2_t", [128, 64]), "u": dram_in("A2_u", [128, 32]),
          "m": dram_in("A2_m", [128, 32])}
    Wk1 = dram_in("Wk1", [128, 128]); bk1 = dram_in("bk1", [128, 1])
    Wk2 = dram_in("Wk2", [128, 128]); bk2 = dram_in("bk2", [128, 1])
    qn1 = dram_in("qn1", [128, 1]); qn2 = dram_in("qn2", [128, 1])
    Wc = dram_in("Wc", [128, 2]); bc = dram_in("bc", [2, 1])
    iota_in = dram_in("iota", [128, 128])
    ident_in = dram_in("ident", [128, 128])
    ones_row_in = dram_in("ones_row", [1, 128])

    yT = nc.dram_tensor("y_T", [2, StP], FP, kind="ExternalOutput")

    with tile.TileContext(nc) as tc:
        import contextlib
        ctx = contextlib.ExitStack()
        with ctx:
            singles = ctx.enter_context(tc.tile_pool(name="singles", bufs=1))
            dram = ctx.enter_context(tc.tile_pool(name="dram", bufs=1, space="DRAM"))

            # load all constants once
            def const(src, shape):
                t = singles.tile(list(shape), FP, tag=f"c_{src.name}")
                nc.sync.dma_start(out=t[:], in_=src[:])
                return t

            W1s = {k: const(W1[k], [128, 128]) for k in "tum"}
            b1s = {k: const(b1[k], [128, 1]) for k in "tum"}
            A1s = {k: const(A1[k], A1[k].shape) for k in "tum"}
            W2s = {k: const(W2[k], [128, 128]) for k in "tum"}
            b2s = {k: const(b2[k], [128, 1]) for k in "tum"}
            A2s = {k: const(A2[k], A2[k].shape) for k in "tum"}
            Wk1s = const(Wk1, [128, 128]); bk1s = const(bk1, [128, 1])
            Wk2s = const(Wk2, [128, 128]); bk2s = const(bk2, [128, 1])
            qn1s = const(qn1, [128, 1]); qn2s = const(qn2, [128, 1])
            Wcs = const(Wc, [128, 2]); bcs = const(bc, [2, 1])
            iota_s = const(iota_in, [128, 128])
            ident_s = const(ident_in, [128, 128])
            ones_r = const(ones_row_in, [1, 128])

            ksum = singles.tile([128, 4], FP)  # cols: ut1, mt1, ut2, mt2
            nc.vector.memset(ksum[:], 0.0)

            # ---- intermediates in DRAM (raw tensors: gathers need offset 0) ----
            zaug = {
                "t": nc.dram_tensor("zaug_t", [NtP, 144], FP),
                "u": nc.dram_tensor("zaug_u", [NuP, 136], FP),
                "m": nc.dram_tensor("zaug_m", [NmP, 136], FP),
            }
            ad1 = {
                "ut": nc.dram_tensor("ad1_ut", [NtP, 8], FP),
                "mt": nc.dram_tensor("ad1_mt", [NtP, 8], FP),
                "tu": nc.dram_tensor("ad1_tu", [NuP, 8], FP),
                "tm": nc.dram_tensor("ad1_tm", [NmP, 8], FP),
            }
            O1 = {
                "ut": nc.dram_tensor("O1_ut", [StP, 128], FP),
                "mt": nc.dram_tensor("O1_mt", [StP, 128], FP),
                "tu": nc.dram_tensor("O1_tu", [SuP, 128], FP),
                "tm": nc.dram_tensor("O1_tm", [SmP, 128], FP),
            }
            z2sh = {"u": dram.tile([SuP, 136], FP, tag="z2sh_u", name="z2sh_u"),
                    "m": dram.tile([SmP, 136], FP, tag="z2sh_m", name="z2sh_m")}
            z2full = {
                "u": nc.dram_tensor("z2full_u", [NCORES * SuP, 136], FP,
                                    addr_space="Shared"),
                "m": nc.dram_tensor("z2full_m", [NCORES * SmP, 136], FP,
                                    addr_space="Shared"),
            }
            ad2 = {"ut": nc.dram_tensor("ad2_ut", [StP, 8], FP),
                   "mt": nc.dram_tensor("ad2_mt", [StP, 8], FP)}
            O2 = {"ut": nc.dram_tensor("O2_ut", [StP, 128], FP),
                  "mt": nc.dram_tensor("O2_mt", [StP, 128], FP)}
            cc_in1 = dram.tile([128, 2], FP, tag="cc_in1", name="cc_in1")
            cc_out1 = dram.tile([128, 2], FP, tag="cc_out1", name="cc_out1")
            cc_in2 = dram.tile([128, 2], FP, tag="cc_in2", name="cc_in2")
            cc_out2 = dram.tile([128, 2], FP, tag="cc_out2", name="cc_out2")

            # ================= P1: replicated projections =================
            with tc.tile_pool(name="p1sb", bufs=3) as sb, \
                 tc.tile_pool(name="p1ps", bufs=2, space="PSUM") as ps, \
                 tc.tile_pool(name="p1ps2", bufs=2, space="PSUM") as ps2:
                for k, NP in (("t", NtP), ("u", NuP), ("m", NmP)):
                    nset = {"t": 4, "u": 2, "m": 2}[k]
                    for i in range(NP // 512):
                        r0 = i * 512
                        xt = sb.tile([128, 512], FP, tag="xt")
                        nc.sync.dma_start(out=xt[:], in_=xT[k][:, r0:r0 + 512])
                        zp = ps.tile([128, 512], FP, tag="zp")
                        nc.tensor.matmul(zp[:], lhsT=W1s[k][:], rhs=xt[:],
                                         start=True, stop=True)
                        zs = sb.tile([128, 512], FP, tag="zs")
                        nc.vector.tensor_scalar_add(zs[:], zp[:], b1s[k][:, 0:1])
                        sp = ps2.tile([128, 512], FP, tag="sp")
                        nc.tensor.matmul(sp[:nset * 32, :], lhsT=A1s[k][:],
                                         rhs=zs[:], start=True, stop=True)
                        ss = sb.tile([128, 512], FP, tag="ss")
                        nc.scalar.copy(ss[:nset * 32, :], sp[:nset * 32, :])
                        rows = zaug[k][r0:r0 + 512, :]
                        _store_rows(nc, zs, 512, rows, ident_s, sb, ps)
                        nas = {"t": 2, "u": 1, "m": 1}[k]
                        for j in range(nas):
                            _store_rows(nc, ss[j * 32:j * 32 + 8, :], 512,
                                        zaug[k][r0:r0 + 512,
                                                128 + j * 8:136 + j * 8],
                                        ident_s, sb, ps)
                        adl = {"t": ("ut", "mt"), "u": ("tu",), "m": ("tm",)}[k]
                        for j, et in enumerate(adl):
                            b0 = (nas + j) * 32
                            _store_rows(nc, ss[b0:b0 + 8, :],
                                        512, ad1[et][r0:r0 + 512, :],
                                        ident_s, sb, ps)

            # ================= edge phase =================
            def edge_phase(et, mdram, sched, src_aug, augw, as_off, ad_arr,
                           ad_col, src_col, O_out, kW, kb, kcol):
                with tc.tile_pool(name=f"e_{et}", bufs=4) as sb, \
                     tc.tile_pool(name=f"ep_{et}", bufs=2, space="PSUM") as ps, \
                     tc.tile_pool(name=f"ep2_{et}", bufs=2, space="PSUM") as ps2:
                    for (bi, ch0, nch) in sched:
                        mt_sb = sb.tile([128, 8 * max(nch, 1)], I32, tag="meta")
                        nc.sync.dma_start(
                            out=mt_sb[:, :8 * nch].rearrange(
                                "p (k c) -> p k c", c=8),
                            in_=mdram[ch0 * 128:(ch0 + nch) * 128, :].rearrange(
                                "(k p) c -> p k c", p=128),
                        )
                        pblk = ps.tile([128, 136], FP, tag="pblk")
                        for kk in range(nch):
                            mc = mt_sb[:, kk * 8:kk * 8 + 8]
                            ze = sb.tile([128, 144], FP, tag="ze")
                            nc.gpsimd.indirect_dma_start(
                                out=ze[:, :augw], out_offset=None,
                                in_=src_aug[:, :],
                                in_offset=bass.IndirectOffsetOnAxis(
                                    ap=mc[:, src_col:src_col + 1], axis=0),
                            )
                            ade = sb.tile([128, 8], FP, tag="ade")
                            nc.gpsimd.indirect_dma_start(
                                out=ade[:], out_offset=None, in_=ad_arr[:, :],
                                in_offset=bass.IndirectOffsetOnAxis(
                                    ap=mc[:, ad_col:ad_col + 1], axis=0),
                            )
                            relf = sb.tile([128, 1], FP, tag="relf")
                            nc.vector.tensor_copy(relf[:], mc[:, 2:3])
                            st = sb.tile([128, 8], FP, tag="st")
                            nc.vector.tensor_add(st[:], ze[:, as_off:as_off + 8],
                                                 ade[:])
                            lr = sb.tile([128, 8], FP, tag="lr")
                            nc.vector.tensor_scalar_mul(lr[:], st[:], 0.2)
                            nc.vector.tensor_tensor(out=lr[:], in0=lr[:],
                                                    in1=st[:], op=ALU.max)
                            wzw = sb.tile([128, 136], FP, tag="wzw")
                            nc.scalar.activation(wzw[:, 128:136], lr[:], AF.Exp)
                            _hmul(nc, wzw[:, 0:128], ze[:, 0:128],
                                  wzw[:, 128:136], sb)
                            sel = sb.tile([128, 128], FP, tag="sel")
                            nc.vector.tensor_tensor(
                                out=sel[:], in0=relf[:, 0:1].to_broadcast((128, 128)),
                                in1=iota_s[:], op=ALU.is_equal)
                            nc.tensor.matmul(pblk[:], lhsT=sel[:], rhs=wzw[:],
                                             start=(kk == 0), stop=(kk == nch - 1))
                        den = sb.tile([128, 8], FP, tag="den")
                        nc.vector.tensor_scalar_add(den[:], pblk[:, 128:136], 1e-16)
                        rd = sb.tile([128, 8], FP, tag="rd")
                        nc.vector.reciprocal(rd[:], den[:])
                        ob = sb.tile([128, 128], FP, tag="ob")
                        nc.scalar.activation(ob[:], pblk[:, 0:128], AF.Relu)
                        _hmul(nc, ob[:], ob[:], rd[:], sb)
                        nc.sync.dma_start(out=O_out[bi * 128:(bi + 1) * 128, :],
                                          in_=ob[:])
                        if kW is not None:
                            tp = ps2.tile([128, 128], FP, tag="ktp")
                            nc.tensor.transpose(tp[:], ob[:], ident_s[:])
                            tsb = sb.tile([128, 128], FP, tag="ktsb")
                            nc.scalar.copy(tsb[:], tp[:])
                            kp = ps2.tile([128, 128], FP, tag="kp")
                            nc.tensor.matmul(kp[:], lhsT=kW[:], rhs=tsb[:],
                                             start=True, stop=True)
                            kt = sb.tile([128, 128], FP, tag="kt")
                            nc.scalar.activation(kt[:], kp[:], AF.Tanh,
                                                 bias=kb[:, 0:1])
                            r1 = sb.tile([128, 1], FP, tag="r1")
                            nc.vector.reduce_sum(r1[:], kt[:],
                                                 axis=mybir.AxisListType.X)
                            nc.vector.tensor_add(ksum[:, kcol:kcol + 1],
                                                 ksum[:, kcol:kcol + 1], r1[:])

            edge_phase("ut", meta["ut"], scheds["ut"][0], zaug["u"], 136, 128,
                       ad1["ut"], 1, 0, O1["ut"], Wk1s, bk1s, 0)
            edge_phase("mt", meta["mt"], scheds["mt"][0], zaug["m"], 136, 128,
                       ad1["mt"], 1, 0, O1["mt"], Wk1s, bk1s, 1)
            edge_phase("tu", meta["tu"], scheds["tu"][0], zaug["t"], 144, 128,
                       ad1["tu"], 1, 0, O1["tu"], None, None, None)
            edge_phase("tm", meta["tm"], scheds["tm"][0], zaug["t"], 144, 136,
                       ad1["tm"], 1, 0, O1["tm"], None, None, None)

            # ================= beta (semantic attention weights) ============
            def beta_block(kc0, kc1, bkc, qnc, cin, cout, tag):
                with tc.tile_pool(name=f"b_{tag}", bufs=1) as sb, \
                     tc.tile_pool(name=f"bp_{tag}", bufs=1, space="PSUM") as ps:
                    tb = sb.tile([128, 1], FP, tag="tb")
                    nc.scalar.activation(tb[:], bkc[:, 0:1], AF.Tanh)
                    S2 = sb.tile([128, 2], FP, tag="S2")
                    for j, kc in enumerate((kc0, kc1)):
                        nc.vector.tensor_scalar_mul(S2[:, j:j + 1], tb[:],
                                                    -float(pad_t))
                        nc.vector.tensor_add(S2[:, j:j + 1], S2[:, j:j + 1],
                                             ksum[:, kc:kc + 1])
                    nc.sync.dma_start(out=cin[:], in_=S2[:])
                    nc.gpsimd.collective_compute(
                        "AllReduce", ALU.add,
                        replica_groups=[list(range(NCORES))],
                        ins=[cin.opt()], outs=[cout.opt()])
                    Ssb = sb.tile([128, 2], FP, tag="Ssb")
                    nc.sync.dma_start(out=Ssb[:], in_=cout[:])
                    scp = ps.tile([1, 2], FP, tag="scp")
                    nc.tensor.matmul(scp[:], lhsT=qnc[:, 0:1], rhs=Ssb[:],
                                     start=True, stop=True)
                    esc = sb.tile([1, 2], FP, tag="esc")
                    nc.scalar.activation(esc[:], scp[:], AF.Exp)
                    sm = sb.tile([1, 1], FP, tag="sm")
                    nc.vector.reduce_sum(sm[:], esc[:], axis=mybir.AxisListType.X)
                    rs = sb.tile([1, 1], FP, tag="rs")
                    nc.vector.reciprocal(rs[:], sm[:])
                    bt = sb.tile([1, 2], FP, tag="bt")
                    nc.vector.tensor_scalar_mul(bt[:], esc[:], rs[:, 0:1])
                    bbp = ps.tile([128, 2], FP, tag="bbp")
                    nc.tensor.matmul(bbp[:], lhsT=ones_r[:], rhs=bt[:],
                                     start=True, stop=True)
                    bb = singles.tile([128, 2], FP, tag=f"bb_{tag}")
                    nc.scalar.copy(bb[:], bbp[:])
                    return bb

            bb1 = beta_block(0, 1, bk1s, qn1s, cc_in1, cc_out1, "l1")

            # ================= P3a: u/m -> z2 shard =================
            with tc.tile_pool(name="p3a", bufs=3) as sb, \
                 tc.tile_pool(name="p3ap", bufs=2, space="PSUM") as ps:
                for k, SP, lo in (("u", SuP, Su), ("m", SmP, Sm)):
                    et = {"u": "tu", "m": "tm"}[k]
                    core_base = None  # shard rows are local
                    for i in range(SP // 512):
                        r0 = i * 512
                        nw = 512
                        ot = _load_cols(nc, O1[et][r0:r0 + nw, :], nw, 128,
                                        sb, ps, ident_s, f"otT_{k}", "oT_tp")
                        zp = ps.tile([128, 512], FP, tag="z2p")
                        nc.tensor.matmul(zp[:, :nw], lhsT=W2s[k][:],
                                         rhs=ot[:, :nw], start=True, stop=True)
                        zs = sb.tile([128, 512], FP, tag="z2s")
                        nc.vector.tensor_scalar_add(zs[:, :nw], zp[:, :nw],
                                                    b2s[k][:, 0:1])
                        sp = ps.tile([32, 512], FP, tag="s2p")
                        nc.tensor.matmul(sp[:, :nw], lhsT=A2s[k][:], rhs=zs[:, :nw],
                                         start=True, stop=True)
                        ss = sb.tile([32, 512], FP, tag="s2s")
                        nc.scalar.copy(ss[:, :nw], sp[:, :nw])
                        _store_rows(nc, zs, nw, z2sh[k][r0:r0 + nw, 0:128],
                                    ident_s, sb, ps)
                        _store_rows(nc, ss[0:8, :], nw,
                                    z2sh[k][r0:r0 + nw, 128:136],
                                    ident_s, sb, ps)

            for k in ("u", "m"):
                nc.gpsimd.collective_compute(
                    "AllGather", ALU.bypass,
                    replica_groups=[list(range(NCORES))],
                    ins=[z2sh[k].opt()], outs=[z2full[k][:, :]])

            # ================= P3c: t combine + project -> ad2 ============
            with tc.tile_pool(name="p3c", bufs=3) as sb, \
                 tc.tile_pool(name="p3cp", bufs=2, space="PSUM") as ps:
                for i in range(StP // 512):
                    r0 = i * 512
                    nw = 512
                    o_ut = _load_cols(nc, O1["ut"][r0:r0 + nw, :], nw, 128,
                                      sb, ps, ident_s, "cutT", "cT_tp")
                    o_mt = _load_cols(nc, O1["mt"][r0:r0 + nw, :], nw, 128,
                                      sb, ps, ident_s, "cmtT", "cT_tp")
                    h1 = sb.tile([128, 512], FP, tag="h1T")
                    nc.vector.tensor_scalar_mul(h1[:, :nw], o_ut[:, :nw],
                                                bb1[:, 0:1])
                    t2 = sb.tile([128, 512], FP, tag="t2T")
                    nc.vector.tensor_scalar_mul(t2[:, :nw], o_mt[:, :nw],
                                                bb1[:, 1:2])
                    nc.vector.tensor_add(h1[:, :nw], h1[:, :nw], t2[:, :nw])
                    zp = ps.tile([128, 512], FP, tag="z2tp")
                    nc.tensor.matmul(zp[:, :nw], lhsT=W2s["t"][:], rhs=h1[:, :nw],
                                     start=True, stop=True)
                    zs = sb.tile([128, 512], FP, tag="z2ts")
                    nc.vector.tensor_scalar_add(zs[:, :nw], zp[:, :nw],
                                                b2s["t"][:, 0:1])
                    sp = ps.tile([64, 512], FP, tag="s2tp")
                    nc.tensor.matmul(sp[:, :nw], lhsT=A2s["t"][:], rhs=zs[:, :nw],
                                     start=True, stop=True)
                    ss = sb.tile([64, 512], FP, tag="s2ts")
                    nc.scalar.copy(ss[:, :nw], sp[:, :nw])
                    _store_rows(nc, ss[0:8, :], nw, ad2["ut"][r0:r0 + nw, :],
                                ident_s, sb, ps)
                    _store_rows(nc, ss[32:40, :], nw, ad2["mt"][r0:r0 + nw, :],
                                ident_s, sb, ps)

            # ================= P4: L2 edge phase =================
            edge_phase("ut2", meta["ut"], scheds["ut"][0], z2full["u"], 136, 128,
                       ad2["ut"], 3, 4, O2["ut"], Wk2s, bk2s, 2)
            edge_phase("mt2", meta["mt"], scheds["mt"][0], z2full["m"], 136, 128,
                       ad2["mt"], 3, 4, O2["mt"], Wk2s, bk2s, 3)

            bb2 = beta_block(2, 3, bk2s, qn2s, cc_in2, cc_out2, "l2")

            # ================= P5: combine + classifier =================
            with tc.tile_pool(name="p5", bufs=3) as sb, \
                 tc.tile_pool(name="p5p", bufs=2, space="PSUM") as ps:
                for i in range(StP // 512):
                    r0 = i * 512
                    nw = 512
                    o_ut = _load_cols(nc, O2["ut"][r0:r0 + nw, :], nw, 128,
                                      sb, ps, ident_s, "futT", "fT_tp")
                    o_mt = _load_cols(nc, O2["mt"][r0:r0 + nw, :], nw, 128,
                                      sb, ps, ident_s, "fmtT", "fT_tp")
                    ht = sb.tile([128, 512], FP, tag="htT")
                    nc.vector.tensor_scalar_mul(ht[:, :nw], o_ut[:, :nw],
                                                bb2[:, 0:1])
                    t2 = sb.tile([128, 512], FP, tag="ft2")
                    nc.vector.tensor_scalar_mul(t2[:, :nw], o_mt[:, :nw],
                                                bb2[:, 1:2])
                    nc.vector.tensor_add(ht[:, :nw], ht[:, :nw], t2[:, :nw])
                    yp = ps.tile([2, 512], FP, tag="yp")
                    nc.tensor.matmul(yp[:, :nw], lhsT=Wcs[:], rhs=ht[:, :nw],
                                     start=True, stop=True)
                    ys = sb.tile([2, 512], FP, tag="ys")
                    nc.vector.tensor_scalar_add(ys[:, :nw], yp[:, :nw],
                                                bcs[:, 0:1])
                    nc.sync.dma_start(out=yT[:, r0:r0 + nw], in_=ys[:, :nw])

    return din


def _prep(inputs, cfg):
    Nt, Nu, Nm = cfg["Nt"], cfg["Nu"], cfg["Nm"]
    St, Su, Sm = Nt // NCORES, Nu // NCORES, Nm // NCORES
    NtP, NuP, NmP = _padblk(Nt), _padblk(Nu), _padblk(Nm)
    SuP, SmP = _padblk(Su), _padblk(Sm)

    p = inputs["params"]
    def npa(x):
        return np.asarray(x, dtype=np.float32)

    metas_ut, sched_ut = _sched_edges(np.asarray(inputs["ei_ut_src"]),
                                      np.asarray(inputs["ei_ut_dst"]), St, NCORES)
    metas_mt, sched_mt = _sched_edges(np.asarray(inputs["ei_mt_src"]),
                                      np.asarray(inputs["ei_mt_dst"]), St, NCORES)
    metas_tu, sched_tu = _sched_edges(np.asarray(inputs["ei_tu_src"]),
                                      np.asarray(inputs["ei_tu_dst"]), Su, NCORES)
    metas_tm, sched_tm = _sched_edges(np.asarray(inputs["ei_tm_src"]),
                                      np.asarray(inputs["ei_tm_dst"]), Sm, NCORES)
    # L2 src remap into padded-shard AllGather layout
    metas_ut[:, :, 4] = (metas_ut[:, :, 0] // Su) * SuP + metas_ut[:, :, 0] % Su
    metas_mt[:, :, 4] = (metas_mt[:, :, 0] // Sm) * SmP + metas_mt[:, :, 0] % Sm

    scheds = {
        "ut": (sched_ut, metas_ut.shape[1] // 128),
        "mt": (sched_mt, metas_mt.shape[1] // 128),
        "tu": (sched_tu, metas_tu.shape[1] // 128),
        "tm": (sched_tm, metas_tm.shape[1] // 128),
    }
    scheds_b = {k: (v[0], v[1] * 128) for k, v in scheds.items()}

    def padT(x, NP):
        x = npa(x)
        out = np.zeros((128, NP), np.float32)
        out[:, :x.shape[0]] = x.T
        return out

    l1, l2 = p["l1"], p["l2"]
    ntmap = {"t": "transaction", "u": "user", "m": "merchant"}
    consts = {}
    consts["iota"] = np.tile(np.arange(128, dtype=np.float32), (128, 1))
    consts["ident"] = np.eye(128, dtype=np.float32)
    consts["ones_row"] = np.ones((1, 128), np.float32)
    for k in "tum":
        W, b = l1["proj"][ntmap[k]]
        consts[f"W1_{k}"] = npa(W)
        consts[f"b1_{k}"] = npa(b).reshape(128, 1)
        W, b = l2["proj"][ntmap[k]]
        consts[f"W2_{k}"] = npa(W)
        consts[f"b2_{k}"] = npa(b).reshape(128, 1)
    # Aseg layer1: per node type [as sets..., ad sets...]
    a1 = {et: (npa(l1["att"][et][0]), npa(l1["att"][et][1]))
          for et in ("ut", "tu", "mt", "tm")}
    a2 = {et: (npa(l2["att"][et][0]), npa(l2["att"][et][1]))
          for et in ("ut", "mt")}
    consts["A1_t"] = _aseg([a1["tu"][0], a1["tm"][0], a1["ut"][1], a1["mt"][1]])
    consts["A1_u"] = _aseg([a1["ut"][0], a1["tu"][1]])
    consts["A1_m"] = _aseg([a1["mt"][0], a1["tm"][1]])
    consts["A2_t"] = _aseg([a2["ut"][1], a2["mt"][1]])
    consts["A2_u"] = _aseg([a2["ut"][0]])
    consts["A2_m"] = _aseg([a2["mt"][0]])
    consts["Wk1"] = npa(l1["k_lin"][0])
    consts["bk1"] = npa(l1["k_lin"][1]).reshape(128, 1)
    consts["Wk2"] = npa(l2["k_lin"][0])
    consts["bk2"] = npa(l2["k_lin"][1]).reshape(128, 1)
    consts["qn1"] = (npa(l1["q"]) / float(Nt)).reshape(128, 1)
    consts["qn2"] = (npa(l2["q"]) / float(Nt)).reshape(128, 1)
    consts["Wc"] = npa(p["cls"][0])
    consts["bc"] = npa(p["cls"][1]).reshape(2, 1)

    in_maps = []
    for c in range(NCORES):
        m = dict(consts)
        m["xT_t"] = padT(inputs["x_transaction"], NtP)
        m["xT_u"] = padT(inputs["x_user"], NuP)
        m["xT_m"] = padT(inputs["x_merchant"], NmP)
        m["meta_ut"] = metas_ut[c]
        m["meta_mt"] = metas_mt[c]
        m["meta_tu"] = metas_tu[c]
        m["meta_tm"] = metas_tm[c]
        in_maps.append(m)
    return in_maps, scheds_b


def _run(inputs, cfg):
    in_maps, scheds = _prep(inputs, cfg)
    nc = bacc.Bacc("TRN2", target_bir_lowering=False, debug=False,
                   num_devices=NCORES)
    build_program(nc, cfg, scheds)
    nc.compile()
    res = bass_utils.run_bass_kernel_spmd(nc, in_maps,
                                          core_ids=list(range(NCORES)))
    Nt = cfg["Nt"]
    St = Nt // NCORES
    out = np.empty((Nt, 2), np.float32)
    for c in range(NCORES):
        out[c * St:(c + 1) * St] = res.results[c]["y_T"][:, :St].T
    return out


def kernel(**inputs) -> np.ndarray:
    cfg = dict(Nt=200000, Nu=50000, Nm=20000)
    return _run(inputs, cfg)
